# revision 1
# baseline (speedup 1.0000x reference)
import numpy as np
import jax
import jax.numpy as jnp
from functools import partial

# nn_DPSTCN: hardcoded problem shapes
B, N, L, D, H, GOUT = 256, 307, 12, 16, 8, 32
M = 8           # cores
BC = B // M     # 32 batches per core


def _pos_encoding():
    pos = np.arange(L, dtype=np.float32)[:, None]
    div = np.power(10000.0, np.arange(0, D, 2, dtype=np.float32) / D)
    ang = pos / div
    P = np.zeros((L, D), dtype=np.float32)
    P[:, 0::2] = np.sin(ang)
    P[:, 1::2] = np.cos(ang)
    return P  # [L, D]


def _core_fn(flow_x, day_g, week_g, his, adj, pe,
             Wq, bq, Wk, bk, Wv, bv, Wo, bo, Wg, Wt, bg, W1, b1, W2, b2):
    # flow_x: [BC, N, L] shard; his: [N, 11+B] replicated (host all-gather of
    # flow_x[:, :, -1] + flow_x[0] per the sharding hint); day_g/week_g:
    # embedding rows gathered by index on host (pure data movement), added here.
    hd = D // H
    sq = jnp.sum(his * his, axis=1)
    d2 = sq[:, None] + sq[None, :] - 2.0 * (his @ his.T)
    fun_graph = jnp.sqrt(jnp.maximum(d2, 0.0))           # [N, N]

    te = day_g + week_g                                   # [BC, L, D]
    x_t = flow_x[..., None] + pe[None, None] + te[:, None]  # [BC, N, L, D]

    def heads(x, W, b):
        return (x @ W + b).reshape(x.shape[0], x.shape[1], L, H, hd)
    q, k, v = heads(x_t, Wq, bq), heads(x_t, Wk, bk), heads(x_t, Wv, bv)
    logits = jnp.einsum('bnlhd,bnmhd->bnhlm', q, k) / jnp.sqrt(jnp.float32(hd))
    att = jnp.einsum('bnhlm,bnmhd->bnlhd', jax.nn.softmax(logits, axis=-1), v)
    att = att.reshape(flow_x.shape[0], N, L, D) @ Wo + bo
    x_tcn = x_t + att

    A_dyn = jax.nn.softmax(-fun_graph, axis=-1)
    A_st = adj / (jnp.sum(adj, axis=-1, keepdims=True) + 1.0)
    x_gcn = flow_x[..., None]
    hid = jax.nn.relu(
        jnp.einsum('nm,bmlc->bnlc', A_dyn, x_gcn @ Wg)
        + jnp.einsum('nm,bmlc->bnlc', A_st, x_tcn @ Wt)
        + bg)

    h1 = jax.nn.relu(jnp.einsum('bnlc,nco->bnlo', hid, W1) + b1[None, :, None])
    out = jnp.einsum('bnlo,noz->bnlz', h1, W2) + b2[None, :, None]
    return out[..., 0]                                    # [BC, N, L]


_pmapped = None


def _get_pmapped():
    global _pmapped
    if _pmapped is None:
        in_axes = (0, 0, 0) + (None,) * 18
        _pmapped = jax.pmap(_core_fn, in_axes=in_axes,
                            devices=jax.devices()[:M])
    return _pmapped


def kernel(flow_x, day_cyc, week_cyc, adj, day_emb, week_emb,
           Wq, bq, Wk, bk, Wv, bv, Wo, bo, Wg, Wt, bg, W1, b1, W2, b2):
    flow_x = np.asarray(flow_x, dtype=np.float32)
    adj = np.asarray(adj, dtype=np.float32)
    day_i = np.asarray(day_cyc).astype(np.int64)
    week_i = np.asarray(week_cyc).astype(np.int64)

    # Host-side data movement only: shard over batch, replicate the his
    # window (all-gather of last timesteps), gather embedding rows by index.
    his = np.concatenate([flow_x[0], flow_x[1:, :, -1].T], axis=1)  # [N, 11+B]
    day_g = np.asarray(day_emb, dtype=np.float32)[day_i]    # [B, L, D]
    week_g = np.asarray(week_emb, dtype=np.float32)[week_i]  # [B, L, D]
    pe = _pos_encoding()

    fx_s = flow_x.reshape(M, BC, N, L)
    dg_s = day_g.reshape(M, BC, L, D)
    wg_s = week_g.reshape(M, BC, L, D)

    f32 = lambda x: np.asarray(x, dtype=np.float32)
    args = (fx_s, dg_s, wg_s, his, adj, pe,
            f32(Wq), f32(bq), f32(Wk), f32(bk), f32(Wv), f32(bv),
            f32(Wo), f32(bo), f32(Wg), f32(Wt), f32(bg),
            f32(W1), f32(b1), f32(W2), f32(b2))
    out = _get_pmapped()(*args)                           # [M, BC, N, L]
    return np.asarray(out).reshape(B, N, L).astype(np.float32)
